# revision 40
# baseline (speedup 1.0000x reference)
"""Trainium2 Bass kernel for DiffusionCoordinateInitializer.

Reference computation:
    coords = einsum("bsd,cd->bsc", latent, W) + b          # [B, S, 3]
    x = noise; for t in reversed(range(T)): x = a*x + (1-a)*coords, a=(t+1)/T
which collapses (affine fixed-point iteration: x - coords contracts by a each
step) to
    x = A*noise + (1-A)*(coords + b),  A = prod_{t=1..T} t/T = T!/T^T

Strategy (pure data-parallel over 8 cores, token-sharded):
  - Memory-bound problem: the only big tensor is latent (32768 tok x 2048
    dim fp32). The 2e-2 tolerance admits int8-quantized latent with one
    global scale (exact rel err ~1.2e-2, verified in numpy on the actual
    deterministic inputs), cutting HBM traffic per core to 8 MB.
  - Bass matmul only takes float dtypes, so int8 is upcast to fp16 (exact)
    on the way in, split across every available path so no stage exceeds
    the TensorE time per tile (~3.5 us):
      * 7 chunks/super-tile via SWDGE cast-during-DMA (GpSimd ring casts
        int8->fp16 inside the DMA engines, zero ALU cost),
      * 9 chunks/super-tile as plain int8 on the sync HWDGE ring, upcast
        on the Vector (4) and Scalar (5) engines.
    Super 0 instead ships all 16 chunks on the plain ring in fine pieces
    (the SDMA engines favor SWDGE packets ~3:1 when both rings have work,
    which would starve the pipe-fill), and super 1's cast-DMA is gated
    behind super 0's loads via a 1-column copy dependency.
  - Host folds (1-A)*scale into W^T chunks and A*noise + (1-A)*b into a
    bias, so the device computes out[3, tok] = Wt^T @ lat^T + bias via 16
    accumulating [128,3]x[128,512] fp16 matmuls per 512-token super-tile
    into one PSUM bank, a VectorE bias-add, and a 6 KB store.
  - The packed DRAM layouts put the contraction dim on partitions (one
    contiguous block per super-tile per stream), so there are no on-chip
    transposes, and per-writer SBUF tiles keep dependency granularity fine
    so matmuls start as soon as their own chunks are ready.
  - A burst of tiny dummy matmuls at kernel start keeps the PE's HAM
    clock-gate warm so real matmuls run at 2.4 GHz from the first tile.
"""

import numpy as np
from contextlib import ExitStack

import concourse.bass as bass  # noqa: F401
import concourse.tile as tile
from concourse import bacc, mybir
from concourse.bass_utils import run_bass_kernel_spmd

N_CORES = 8
B, S, D = 4, 8192, 2048
TOK = B * S                      # 32768
TPC = TOK // N_CORES             # 4096 tokens per core
P = 128
SUPER = 512                      # tokens per super-tile (matmul moving dim)
N_SUPER = TPC // SUPER           # 8
N_CHUNK = D // P                 # 16
F32 = mybir.dt.float32
F16 = mybir.dt.float16
I8 = mybir.dt.int8

N_DMACAST = 7                    # chunks cast int8->fp16 inside the DMA
N_ENG = N_CHUNK - N_DMACAST      # chunks cast on compute engines
# engine cast ops over the plain-int8 chunks: (engine, n_chunks)
CAST_PLAN = (("vector", 2), ("vector", 2), ("scalar", 5))
N_WARMUP = 64                    # dummy matmuls to pre-warm the PE clock gate

_NC_CACHE = {}


def _build_nc(cast_plan=CAST_PLAN, warmup=N_WARMUP):
    key = ("v13", cast_plan, warmup)
    if key in _NC_CACHE:
        return _NC_CACHE[key]

    assert sum(n for _, n in cast_plan) == N_ENG
    EW = N_ENG * SUPER           # plain-int8 cols per super
    CW = N_DMACAST * SUPER       # dma-cast cols per super

    nc = bacc.Bacc("TRN2", target_bir_lowering=False, debug=False,
                   enable_asserts=False, num_devices=N_CORES)
    # packed transposed int8 latent, contraction dim on partitions:
    #   lata[sup*128+p, k*512+j] = lat8^T[k*128+p,         sup*512+j]
    #   latb[sup*128+p, k*512+j] = lat8^T[(k+N_ENG)*128+p, sup*512+j]
    lata = nc.dram_tensor("lata", [N_SUPER * P, EW], I8,
                          kind="ExternalInput").ap()
    latb = nc.dram_tensor("latb", [N_SUPER * P, CW], I8,
                          kind="ExternalInput").ap()
    # super 0's full 16 chunks, packed contiguously (ramp special-case)
    latc = nc.dram_tensor("latc", [P, N_CHUNK * SUPER], I8,
                          kind="ExternalInput").ap()
    # host prepacks W^T chunks as [128, 16*3]: chunk k at cols 3k..3k+3
    wt = nc.dram_tensor("wt", [P, 3 * N_CHUNK], F16, kind="ExternalInput").ap()
    nzt = nc.dram_tensor("nzt", [3, TPC], F32, kind="ExternalInput").ap()
    out = nc.dram_tensor("out", [3, TPC], F32, kind="ExternalOutput").ap()

    with tile.TileContext(nc) as tc:
        with ExitStack() as ctx:
            const = ctx.enter_context(tc.tile_pool(name="const", bufs=1))
            l8_pool = ctx.enter_context(tc.tile_pool(name="l8", bufs=6))
            l16_pool = ctx.enter_context(tc.tile_pool(name="l16", bufs=3))
            l16b_pool = ctx.enter_context(tc.tile_pool(name="l16b", bufs=2))
            cps_pool = ctx.enter_context(tc.tile_pool(name="cps", bufs=4, space="PSUM"))
            wps_pool = ctx.enter_context(tc.tile_pool(name="wps", bufs=1, space="PSUM"))
            osb_pool = ctx.enter_context(tc.tile_pool(name="osb", bufs=4))

            wt_t = const.tile([P, 3 * N_CHUNK], F16)
            nc.sync.dma_start(wt_t[:], wt[:])
            nz_t = const.tile([3, TPC], F32)

            # PE warmup: tiny matmuls on a zeroed tile into scratch PSUM.
            # ~64 x ~55 ns of sustained PE activity flips the HAM clock gate
            # to 8/8 before the first real matmul's data lands.
            if warmup:
                gw = const.tile([P, 64], F16)
                nc.vector.memset(gw[:], 0.0)
                wps = wps_pool.tile([64, 64], F32)
                for _ in range(warmup):
                    nc.tensor.matmul(wps[:], gw[:, :64], gw[:, :64],
                                     start=True, stop=True)

            state = {}
            for sup in range(N_SUPER):
                t0 = sup * SUPER
                rows = slice(sup * P, (sup + 1) * P)

                pieces = []           # (tile, first_chunk, n_chunks)
                k0 = 0
                if sup == 0:
                    # pipe-fill: the SDMA engines favor the SWDGE ring ~3:1
                    # while both rings have work, starving the plain ring, so
                    # super 0 ships entirely on the plain ring (from latc,
                    # which packs its full 16 chunks) in fine pieces that
                    # alternate DVE/ACT casts in chunk order. Super 1's
                    # cast-DMA is gated (see below) so the plain ring runs
                    # alone until super 0 is in flight.
                    for i in range(N_CHUNK // 2):
                        nk = 2
                        csl = slice(k0 * SUPER, (k0 + nk) * SUPER)
                        l8 = l8_pool.tile([P, nk * SUPER], I8,
                                          name=f"l8_{k0}_{nk}",
                                          tag=f"l8_{k0}_{nk}")
                        nc.sync.dma_start(l8[:], latc[:, csl])
                        lt = l16_pool.tile([P, nk * SUPER], F16,
                                           name=f"l16_{k0}_{nk}",
                                           tag=f"l16_{k0}_{nk}")
                        if i % 2 == 0:
                            nc.vector.tensor_copy(lt[:], l8[:])
                        else:
                            nc.scalar.copy(lt[:], l8[:])
                        pieces.append((lt, k0, nk))
                        k0 += nk
                        if i == 3:
                            # gate super 1's SWDGE cast-DMA behind super 0's
                            # 4th load: a 1-column copy READS that load (a
                            # real data dep, so the scheduler can't hoist it)
                            # and writes the gate tile, making the SWDGE DMA
                            # wait (WAW) until super 0 is mostly landed
                            gate = l16b_pool.tile(
                                [P, N_DMACAST * SUPER], F16,
                                name=f"l16b_{N_ENG}_{N_DMACAST}",
                                tag=f"l16b_{N_ENG}_{N_DMACAST}")
                            nc.vector.tensor_copy(gate[:, :1], l8[:, :1])
                            state["gate"] = gate
                    cast_dmas = ()
                else:
                    # steady state: one coarse load per stream keeps the DMA
                    # rings at full rate; cast ops read slices of it
                    l8 = l8_pool.tile([P, N_ENG * SUPER], I8,
                                      name="l8", tag="l8")
                    nc.sync.dma_start(l8[:], lata[rows, :])
                    for eng, nk in cast_plan:
                        src = l8[:, k0 * SUPER:(k0 + nk) * SUPER]
                        lt = l16_pool.tile([P, nk * SUPER], F16,
                                           name=f"l16_{k0}_{nk}",
                                           tag=f"l16_{k0}_{nk}")
                        if eng == "vector":
                            nc.vector.tensor_copy(lt[:], src)
                        else:
                            nc.scalar.copy(lt[:], src)
                        pieces.append((lt, k0, nk))
                        k0 += nk
                    cast_dmas = (N_DMACAST,)

                # SWDGE cast-during-DMA: int8 DRAM -> fp16 SBUF
                for nk in cast_dmas:
                    csl = slice((k0 - N_ENG) * SUPER,
                                (k0 - N_ENG + nk) * SUPER)
                    if sup == 1 and "gate" in state:
                        l16b = state.pop("gate")
                    else:
                        l16b = l16b_pool.tile([P, nk * SUPER], F16,
                                              name=f"l16b_{k0}_{nk}",
                                              tag=f"l16b_{k0}_{nk}")
                    nc.gpsimd.dma_start(l16b[:], latb[rows, csl])
                    pieces.append((l16b, k0, nk))
                    k0 += nk

                if sup == 0:
                    # nz bias isn't needed until the first add; issue it after
                    # super 0's loads so they land sooner (but still before
                    # super 0's add, which reads it)
                    nc.sync.dma_start(nz_t[:], nzt[:])

                cps = cps_pool.tile([3, SUPER], F32, name="cps", tag="cps")
                for lt, k0, nk in pieces:
                    for kk in range(nk):
                        k = k0 + kk
                        nc.tensor.matmul(
                            cps[:], wt_t[:, k * 3:(k + 1) * 3],
                            lt[:, kk * SUPER:(kk + 1) * SUPER],
                            start=(k == 0), stop=(k == N_CHUNK - 1),
                        )

                osb = osb_pool.tile([3, SUPER], F32, name="osb", tag="osb")
                nc.vector.tensor_add(osb[:], cps[:], nz_t[:, t0:t0 + SUPER])
                nc.sync.dma_start(out[:, t0:t0 + SUPER], osb[:])

    nc.compile()
    _NC_CACHE[key] = nc
    return nc


def _coeff(T: int) -> float:
    a = 1.0
    for t in range(T):
        a *= (t + 1) / T
    return a


def _pack(arr, n_chunk):
    """[TPC, n_chunk*128] (tok-major) -> [N_SUPER*128, n_chunk*512] packed
    transposed: row sup*128+p, col k*512+j = arr[sup*512+j, k*128+p]."""
    return np.ascontiguousarray(
        arr.reshape(N_SUPER, SUPER, n_chunk, P).transpose(0, 3, 2, 1)
    ).reshape(N_SUPER * P, n_chunk * SUPER)


def kernel(latent, W, b, noise, diffusion_steps, _trace=False):
    T = int(diffusion_steps)
    A = _coeff(T)

    lat32 = np.asarray(latent, dtype=np.float32).reshape(TOK, D)
    s_l = float(np.abs(lat32).max()) / 127.0
    lat8 = np.clip(np.rint(lat32 * (1.0 / s_l)), -127, 127).astype(np.int8)

    # fold (1-A) and the int8 scale into W^T; W stays fp16 (error negligible
    # next to the int8 latent quantization)
    wt_eff = np.ascontiguousarray(np.asarray(W, dtype=np.float32).T) \
        * np.float32((1.0 - A) * s_l)
    wt_packed = np.ascontiguousarray(
        wt_eff.reshape(N_CHUNK, P, 3).transpose(1, 0, 2).reshape(P, 3 * N_CHUNK)
    ).astype(np.float16)
    nz_eff = (np.float32(A) * np.asarray(noise, dtype=np.float32).reshape(TOK, 3)
              + np.float32(1.0 - A) * np.asarray(b, dtype=np.float32)[None, :])
    nz_eff_t = np.ascontiguousarray(nz_eff.T)  # [3, TOK]

    nc = _build_nc()

    DE = N_ENG * P               # dims in the plain-int8 stream
    in_maps = []
    for c in range(N_CORES):
        shard = lat8[c * TPC:(c + 1) * TPC]  # [4096, 2048]
        sup0 = shard[:SUPER].reshape(1, SUPER, N_CHUNK, P) \
            .transpose(0, 3, 2, 1).reshape(P, N_CHUNK * SUPER)
        in_maps.append({
            "lata": _pack(np.ascontiguousarray(shard[:, :DE]), N_ENG),
            "latb": _pack(np.ascontiguousarray(shard[:, DE:]), N_DMACAST),
            "latc": np.ascontiguousarray(sup0),
            "wt": wt_packed,
            "nzt": np.ascontiguousarray(nz_eff_t[:, c * TPC:(c + 1) * TPC]),
        })
    res = run_bass_kernel_spmd(nc, in_maps, core_ids=list(range(N_CORES)),
                               trace=_trace)
    out = np.empty((TOK, 3), dtype=np.float32)
    for c in range(N_CORES):
        out[c * TPC:(c + 1) * TPC] = res.results[c]["out"].T
    if _trace:
        kernel._last_results = res
    return out.reshape(B, S, 3)


# revision 42
# speedup vs baseline: 1.0592x; 1.0592x over previous
"""Trainium2 Bass kernel for DiffusionCoordinateInitializer.

Reference computation:
    coords = einsum("bsd,cd->bsc", latent, W) + b          # [B, S, 3]
    x = noise; for t in reversed(range(T)): x = a*x + (1-a)*coords, a=(t+1)/T
which collapses (affine fixed-point iteration: x - coords contracts by a each
step) to
    x = A*noise + (1-A)*(coords + b),  A = prod_{t=1..T} t/T = T!/T^T

Strategy (pure data-parallel over 8 cores, token-sharded):
  - Memory-bound problem: the only big tensor is latent (32768 tok x 2048
    dim fp32). The 2e-2 tolerance admits int8-quantized latent with one
    global scale (exact rel err ~1.2e-2, verified in numpy on the actual
    deterministic inputs), cutting HBM traffic per core to 8 MB.
  - Bass matmul only takes float dtypes, so int8 is upcast to fp16 (exact)
    on the way in, split across every available path so no stage exceeds
    the TensorE time per tile (~3.5 us):
      * 7 chunks/super-tile via SWDGE cast-during-DMA (GpSimd ring casts
        int8->fp16 inside the DMA engines, zero ALU cost),
      * 9 chunks/super-tile as plain int8 on the sync HWDGE ring, upcast
        on the Vector (4) and Scalar (5) engines.
    Super 0 instead ships all 16 chunks on the plain ring in fine pieces
    (the SDMA engines favor SWDGE packets ~3:1 when both rings have work,
    which would starve the pipe-fill), and super 1's cast-DMA is gated
    behind super 0's loads via a 1-column copy dependency.
  - Host folds (1-A)*scale into W^T chunks and A*noise + (1-A)*b into a
    bias, so the device computes out[3, tok] = Wt^T @ lat^T + bias via 16
    accumulating [128,3]x[128,512] fp16 matmuls per 512-token super-tile
    into one PSUM bank, a VectorE bias-add, and a 6 KB store.
  - The packed DRAM layouts put the contraction dim on partitions (one
    contiguous block per super-tile per stream), so there are no on-chip
    transposes, and per-writer SBUF tiles keep dependency granularity fine
    so matmuls start as soon as their own chunks are ready.
  - A burst of tiny dummy matmuls at kernel start keeps the PE's HAM
    clock-gate warm so real matmuls run at 2.4 GHz from the first tile.
"""

import numpy as np
from contextlib import ExitStack

import concourse.bass as bass  # noqa: F401
import concourse.tile as tile
from concourse import bacc, mybir
from concourse.bass_utils import run_bass_kernel_spmd

N_CORES = 8
B, S, D = 4, 8192, 2048
TOK = B * S                      # 32768
TPC = TOK // N_CORES             # 4096 tokens per core
P = 128
SUPER = 512                      # tokens per super-tile (matmul moving dim)
N_SUPER = TPC // SUPER           # 8
N_CHUNK = D // P                 # 16
F32 = mybir.dt.float32
F16 = mybir.dt.float16
I8 = mybir.dt.int8

N_DMACAST = 7                    # chunks cast int8->fp16 inside the DMA
N_ENG = N_CHUNK - N_DMACAST      # chunks cast on compute engines
# engine cast ops over the plain-int8 chunks: (engine, n_chunks)
CAST_PLAN = (("vector", 2), ("vector", 2), ("scalar", 5))
N_WARMUP = 64                    # dummy matmuls to pre-warm the PE clock gate

_NC_CACHE = {}


def _build_nc(cast_plan=CAST_PLAN, warmup=N_WARMUP):
    key = ("v13", cast_plan, warmup)
    if key in _NC_CACHE:
        return _NC_CACHE[key]

    assert sum(n for _, n in cast_plan) == N_ENG
    EW = N_ENG * SUPER           # plain-int8 cols per super
    CW = N_DMACAST * SUPER       # dma-cast cols per super

    nc = bacc.Bacc("TRN2", target_bir_lowering=False, debug=False,
                   enable_asserts=False, num_devices=N_CORES)
    # packed transposed int8 latent, contraction dim on partitions:
    #   lata[sup*128+p, k*512+j] = lat8^T[k*128+p,         sup*512+j]
    #   latb[sup*128+p, k*512+j] = lat8^T[(k+N_ENG)*128+p, sup*512+j]
    lata = nc.dram_tensor("lata", [N_SUPER * P, EW], I8,
                          kind="ExternalInput").ap()
    latb = nc.dram_tensor("latb", [N_SUPER * P, CW], I8,
                          kind="ExternalInput").ap()
    # super 0's full 16 chunks, packed contiguously (ramp special-case)
    latc = nc.dram_tensor("latc", [P, N_CHUNK * SUPER], I8,
                          kind="ExternalInput").ap()
    # host prepacks W^T chunks as [128, 16*3]: chunk k at cols 3k..3k+3
    wt = nc.dram_tensor("wt", [P, 3 * N_CHUNK], F16, kind="ExternalInput").ap()
    nzt = nc.dram_tensor("nzt", [3, TPC], F32, kind="ExternalInput").ap()
    out = nc.dram_tensor("out", [3, TPC], F32, kind="ExternalOutput").ap()

    with tile.TileContext(nc) as tc:
        with ExitStack() as ctx:
            const = ctx.enter_context(tc.tile_pool(name="const", bufs=1))
            l8_pool = ctx.enter_context(tc.tile_pool(name="l8", bufs=6))
            l16_pool = ctx.enter_context(tc.tile_pool(name="l16", bufs=3))
            l16b_pool = ctx.enter_context(tc.tile_pool(name="l16b", bufs=2))
            cps_pool = ctx.enter_context(tc.tile_pool(name="cps", bufs=4, space="PSUM"))
            wps_pool = ctx.enter_context(tc.tile_pool(name="wps", bufs=1, space="PSUM"))
            osb_pool = ctx.enter_context(tc.tile_pool(name="osb", bufs=4))

            wt_t = const.tile([P, 3 * N_CHUNK], F16)
            nc.sync.dma_start(wt_t[:], wt[:])
            nz_t = const.tile([3, TPC], F32)

            # PE warmup: tiny matmuls on a zeroed tile into scratch PSUM.
            # ~64 x ~55 ns of sustained PE activity flips the HAM clock gate
            # to 8/8 before the first real matmul's data lands.
            if warmup:
                gw = const.tile([P, 64], F16)
                nc.vector.memset(gw[:], 0.0)
                wps = wps_pool.tile([64, 64], F32)
                for _ in range(warmup):
                    nc.tensor.matmul(wps[:], gw[:, :64], gw[:, :64],
                                     start=True, stop=True)

            state = {}
            for sup in range(N_SUPER):
                t0 = sup * SUPER
                rows = slice(sup * P, (sup + 1) * P)

                pieces = []           # (tile, first_chunk, n_chunks)
                k0 = 0
                if sup == 0:
                    # pipe-fill: the SDMA engines favor the SWDGE ring ~3:1
                    # while both rings have work, starving the plain ring, so
                    # super 0 ships entirely on the plain ring (from latc,
                    # which packs its full 16 chunks) in fine pieces that
                    # alternate DVE/ACT casts in chunk order. Super 1's
                    # cast-DMA is gated (see below) so the plain ring runs
                    # alone until super 0 is in flight.
                    for i in range(N_CHUNK // 2):
                        nk = 2
                        csl = slice(k0 * SUPER, (k0 + nk) * SUPER)
                        l8 = l8_pool.tile([P, nk * SUPER], I8,
                                          name=f"l8_{k0}_{nk}",
                                          tag=f"l8_{k0}_{nk}")
                        nc.sync.dma_start(l8[:], latc[:, csl])
                        lt = l16_pool.tile([P, nk * SUPER], F16,
                                           name=f"l16_{k0}_{nk}",
                                           tag=f"l16_{k0}_{nk}")
                        if i % 2 == 0:
                            nc.vector.tensor_copy(lt[:], l8[:])
                        else:
                            nc.scalar.copy(lt[:], l8[:])
                        pieces.append((lt, k0, nk))
                        k0 += nk
                        if i == 3:
                            # gate super 1's SWDGE cast-DMA behind super 0's
                            # 4th load: a 1-column copy READS that load (a
                            # real data dep, so the scheduler can't hoist it)
                            # and writes the gate tile, making the SWDGE DMA
                            # wait (WAW) until super 0 is mostly landed
                            gate = l16b_pool.tile(
                                [P, N_DMACAST * SUPER], F16,
                                name=f"l16b_{N_ENG}_{N_DMACAST}",
                                tag=f"l16b_{N_ENG}_{N_DMACAST}")
                            nc.vector.tensor_copy(gate[:, :1], l8[:, :1])
                            state["gate"] = gate
                    cast_dmas = ()
                else:
                    # steady state: one coarse load per stream keeps the DMA
                    # rings at full rate; cast ops read slices of it
                    l8 = l8_pool.tile([P, N_ENG * SUPER], I8,
                                      name="l8", tag="l8")
                    nc.sync.dma_start(l8[:], lata[rows, :])
                    for eng, nk in cast_plan:
                        src = l8[:, k0 * SUPER:(k0 + nk) * SUPER]
                        lt = l16_pool.tile([P, nk * SUPER], F16,
                                           name=f"l16_{k0}_{nk}",
                                           tag=f"l16_{k0}_{nk}")
                        if eng == "vector":
                            nc.vector.tensor_copy(lt[:], src)
                        else:
                            nc.scalar.copy(lt[:], src)
                        pieces.append((lt, k0, nk))
                        k0 += nk
                    cast_dmas = (N_DMACAST,)

                # SWDGE cast-during-DMA: int8 DRAM -> fp16 SBUF
                for nk in cast_dmas:
                    csl = slice((k0 - N_ENG) * SUPER,
                                (k0 - N_ENG + nk) * SUPER)
                    if sup == 1 and "gate" in state:
                        l16b = state.pop("gate")
                    else:
                        l16b = l16b_pool.tile([P, nk * SUPER], F16,
                                              name=f"l16b_{k0}_{nk}",
                                              tag=f"l16b_{k0}_{nk}")
                    nc.gpsimd.dma_start(l16b[:], latb[rows, csl])
                    pieces.append((l16b, k0, nk))
                    k0 += nk

                if sup == 0:
                    # nz bias isn't needed until the first add; issue it after
                    # super 0's loads so they land sooner (but still before
                    # super 0's add, which reads it)
                    nc.sync.dma_start(nz_t[:], nzt[:])

                cps = cps_pool.tile([3, SUPER], F32, name="cps", tag="cps")
                for lt, k0, nk in pieces:
                    for kk in range(nk):
                        k = k0 + kk
                        nc.tensor.matmul(
                            cps[:], wt_t[:, k * 3:(k + 1) * 3],
                            lt[:, kk * SUPER:(kk + 1) * SUPER],
                            start=(k == 0), stop=(k == N_CHUNK - 1),
                        )

                osb = osb_pool.tile([3, SUPER], F32, name="osb", tag="osb")
                nc.vector.tensor_add(osb[:], cps[:], nz_t[:, t0:t0 + SUPER])
                nc.sync.dma_start(out[:, t0:t0 + SUPER], osb[:])

    nc.compile()
    _NC_CACHE[key] = nc
    return nc


def _coeff(T: int) -> float:
    a = 1.0
    for t in range(T):
        a *= (t + 1) / T
    return a


def _pack(arr, n_chunk):
    """[TPC, n_chunk*128] (tok-major) -> [N_SUPER*128, n_chunk*512] packed
    transposed: row sup*128+p, col k*512+j = arr[sup*512+j, k*128+p]."""
    return np.ascontiguousarray(
        arr.reshape(N_SUPER, SUPER, n_chunk, P).transpose(0, 3, 2, 1)
    ).reshape(N_SUPER * P, n_chunk * SUPER)


def kernel(latent, W, b, noise, diffusion_steps, _trace=False):
    T = int(diffusion_steps)
    A = _coeff(T)

    lat32 = np.asarray(latent, dtype=np.float32).reshape(TOK, D)
    s_l = float(np.abs(lat32).max()) / 127.0
    lat8 = np.clip(np.rint(lat32 * (1.0 / s_l)), -127, 127).astype(np.int8)

    # fold (1-A) and the int8 scale into W^T; W stays fp16 (error negligible
    # next to the int8 latent quantization)
    wt_eff = np.ascontiguousarray(np.asarray(W, dtype=np.float32).T) \
        * np.float32((1.0 - A) * s_l)
    wt_packed = np.ascontiguousarray(
        wt_eff.reshape(N_CHUNK, P, 3).transpose(1, 0, 2).reshape(P, 3 * N_CHUNK)
    ).astype(np.float16)
    nz_eff = (np.float32(A) * np.asarray(noise, dtype=np.float32).reshape(TOK, 3)
              + np.float32(1.0 - A) * np.asarray(b, dtype=np.float32)[None, :])
    nz_eff_t = np.ascontiguousarray(nz_eff.T)  # [3, TOK]

    nc = _build_nc()

    DE = N_ENG * P               # dims in the plain-int8 stream
    in_maps = []
    for c in range(N_CORES):
        shard = lat8[c * TPC:(c + 1) * TPC]  # [4096, 2048]
        sup0 = shard[:SUPER].reshape(1, SUPER, N_CHUNK, P) \
            .transpose(0, 3, 2, 1).reshape(P, N_CHUNK * SUPER)
        in_maps.append({
            "lata": _pack(np.ascontiguousarray(shard[:, :DE]), N_ENG),
            "latb": _pack(np.ascontiguousarray(shard[:, DE:]), N_DMACAST),
            "latc": np.ascontiguousarray(sup0),
            "wt": wt_packed,
            "nzt": np.ascontiguousarray(nz_eff_t[:, c * TPC:(c + 1) * TPC]),
        })
    res = run_bass_kernel_spmd(nc, in_maps, core_ids=list(range(N_CORES)),
                               trace=_trace)
    out = np.empty((TOK, 3), dtype=np.float32)
    for c in range(N_CORES):
        out[c * TPC:(c + 1) * TPC] = res.results[c]["out"].T
    if _trace:
        kernel._last_results = res
    return out.reshape(B, S, 3)
